# revision 1
# baseline (speedup 1.0000x reference)
"""Dense 2-layer GAT (4 heads) on 8 Trainium2 NeuronCores.

Distribution: 1D row-parallel over destination nodes. Core c owns rows
R_c = [512c, 512c+512). Each core computes its rows of both GAT layers;
a mid-kernel AllGather exchanges the layer-2 projections [Wh2 | d2]
(2.1 MB) instead of h1 (the only cross-core dependency).

On-device layout: attention is built TRANSPOSED, att[j, i] (source node j
on partitions, my rows i on free), so
  - hT[o, i] = sum_j Wh[j, o] * att[j, i] needs no transpose of att,
  - softmax denominators come free as an extra ones-column in lhsT,
  - the layer output hT is exactly the lhsT the next layer's projection
    needs.
x is pre-transposed on host (layout-only prep); adj is pre-transposed and
column-sliced per core on host. Output is produced as h2T [128, 512] per
core and un-transposed on host.

Per layer the logits pipeline is:
  v = s_rep + d      (DVE tensor_scalar, 2x mode, per-partition bias d_j)
  v = Prelu(v, 0.2)  (ScalarE, one instruction per [128, 4096] superblock)
  v = Exp(v)         (ScalarE)
  v = v * adjT       (DVE tensor_tensor; adj in {0,1} => exact masking)
followed by 8 accumulating matmuls per superblock into hT psum.
"""
import sys

if "/opt/trn_rl_repo" not in sys.path:
    sys.path.insert(0, "/opt/trn_rl_repo")

import numpy as np

import concourse.bacc as bacc
import concourse.mybir as mybir
import concourse.tile as tile
from concourse.bass_utils import run_bass_kernel_spmd

F32 = mybir.dt.float32
AF = mybir.ActivationFunctionType
OP = mybir.AluOpType

N = 4096
NFEAT = 512
NHID = 256
NEMBED = 128
NHEADS = 4
O1 = 64
O2 = 32
NCORES = 8
R = N // NCORES          # 512 rows per core
ALPHA = 0.2
NT = N // 128            # 32 j-tiles
SB = 8                   # j-tiles per superblock
NSB = NT // SB           # 4 superblocks
BLK1 = O1 + 1            # 65: [Wh_h | ones]
BLK2 = O2 + 1            # 33
GCOLS = NEMBED + NHEADS  # 132: [Wh2 (128) | d2 (4)]


def _build(debug=False, repeat=1):
    nc = bacc.Bacc("TRN2", target_bir_lowering=False, debug=False,
                   num_devices=NCORES)

    xT = nc.dram_tensor("xT", [NFEAT, N], F32, kind="ExternalInput").ap()
    xmT = nc.dram_tensor("xmT", [NFEAT, R], F32, kind="ExternalInput").ap()
    adjT = nc.dram_tensor("adjT", [N, R], F32, kind="ExternalInput").ap()
    W1 = nc.dram_tensor("W1", [NHEADS, NFEAT, O1], F32, kind="ExternalInput").ap()
    a1f = nc.dram_tensor("a1f", [1, 2 * O1 * NHEADS], F32, kind="ExternalInput").ap()
    W2 = nc.dram_tensor("W2", [NHEADS, NHID, O2], F32, kind="ExternalInput").ap()
    a2f = nc.dram_tensor("a2f", [1, 2 * O2 * NHEADS], F32, kind="ExternalInput").ap()
    out = nc.dram_tensor("h2T", [NEMBED, R], F32, kind="ExternalOutput").ap()
    dbg = None
    if debug:
        dbg = {
            "d_srep0": nc.dram_tensor("d_srep0", [128, R], F32, kind="ExternalOutput").ap(),
            "d_WhD1": nc.dram_tensor("d_WhD1", [128, NT, 4 * BLK1 + NHEADS], F32, kind="ExternalOutput").ap(),
            "d_H1T": nc.dram_tensor("d_H1T", [128, 2, R], F32, kind="ExternalOutput").ap(),
            "d_Gall": nc.dram_tensor("d_Gall", [N, GCOLS], F32, kind="ExternalOutput").ap(),
            "d_srep20": nc.dram_tensor("d_srep20", [128, R], F32, kind="ExternalOutput").ap(),
            "d_att0": nc.dram_tensor("d_att0", [128, SB, R], F32, kind="ExternalOutput").ap(),
        }

    with tile.TileContext(nc) as tc:
        for _rep in range(repeat):
            _emit(tc, nc, xT, xmT, adjT, W1, a1f, W2, a2f, out, dbg=dbg)
    nc.compile()
    return nc


def _emit(tc, nc, xT, xmT, adjT, W1, a1f, W2, a2f, out, dbg=None):
    v_ = nc.vector
    s_ = nc.scalar
    t_ = nc.tensor

    with (
        tc.tile_pool(name="persist", bufs=1) as P,
        tc.tile_pool(name="small", bufs=VARIANT.get("sp_bufs", 2)) as SP,
        tc.tile_pool(name="psA", bufs=1, space="PSUM") as PSA,
        tc.tile_pool(name="psB", bufs=VARIANT.get("psb_bufs", 3), space="PSUM") as PSB,
        tc.tile_pool(name="dram", bufs=1, space="DRAM") as DP,
    ):
        ones_row = P.tile([1, 128], F32, tag="ones_row")
        v_.memset(ones_row[:], 1.0)

        # ---- per-head a-vector prep for both layers -------------------
        asrc_rep = {}   # (l, h) -> [Fo, 128] a_src broadcast along free
        adst_rep = {}   # (l, h) -> [128, Fo] a_dst broadcast along partitions
        for l, af, Fo in ((1, a1f, O1), (2, a2f, O2)):
            for h in range(NHEADS):
                col = P.tile([Fo, 1], F32, tag=f"asrc_col{l}_{h}")
                nc.sync.dma_start(col[:], af[0:1, 2 * Fo * h: 2 * Fo * h + Fo])
                rep = P.tile([Fo, 128], F32, tag=f"asrc_rep{l}_{h}")
                v_.memset(rep[:], 0.0)
                s_.activation(rep[:], rep[:], AF.Identity, bias=col[:], scale=0.0)
                asrc_rep[(l, h)] = rep

                row = P.tile([1, Fo], F32, tag=f"adst_row{l}_{h}")
                nc.sync.dma_start(row[:], af[0:1, 2 * Fo * h + Fo: 2 * Fo * h + 2 * Fo])
                dps = PSB.tile([128, Fo], F32, tag="ps")
                t_.matmul(dps[:], ones_row[:], row[:], start=True, stop=True)
                drep = P.tile([128, Fo], F32, tag=f"adst_rep{l}_{h}")
                v_.tensor_copy(drep[:], dps[:])
                adst_rep[(l, h)] = drep

        AP_ctx = tc.tile_pool(name="adj", bufs=VARIANT.get("adj_bufs", 2))
        AP_ = AP_ctx.__enter__()
        VP_ctx = tc.tile_pool(name="vwork", bufs=VARIANT.get("v_bufs", 3))
        VP = VP_ctx.__enter__()
        XP_ctx = tc.tile_pool(name="xload", bufs=1)
        XP = XP_ctx.__enter__()
        # ---- layer-1 front: WR1 = [W1 all heads | w_tilde], then ------
        # WhD1[:, nt, :] = [Wh_h | 1]*4 | d_h*4  via one matmul per (nt, fc)
        WR1 = XP.tile([128, 4, 4 * O1 + NHEADS], F32, tag="WR1")
        for h in range(NHEADS):
            nc.sync.dma_start(
                WR1[:, :, O1 * h: O1 * h + O1],
                W1[h, :, :].rearrange("(c p) o -> p c o", p=128),
            )
        # w_tilde[f, h] = sum_o W1[h][f, o] * a_dst[h][o]
        for h in range(NHEADS):
            for fc in range(4):
                tmp = SP.tile([128, O1], F32, tag="wtld_tmp")
                v_.tensor_tensor(tmp[:], WR1[:, fc, O1 * h: O1 * h + O1],
                                 adst_rep[(1, h)][:], op=OP.mult)
                v_.reduce_sum(WR1[:, fc, 4 * O1 + h: 4 * O1 + h + 1],
                              tmp[:], axis=mybir.AxisListType.X)

        HN = N // VARIANT.get("xt_div", 2)
        XT = XP.tile([128, 4, HN], F32, tag="XT")
        XM = XP.tile([128, 4, R], F32, tag="XM")
        nc.sync.dma_start(XM[:], xmT[:, :].rearrange("(c p) n -> p c n", p=128))

        # ---- s1_rep first: unblocks layer-1 attention early ----------
        s_rep1 = {}
        for h in range(NHEADS):
            wps = PSB.tile([O1, R], F32, tag="ps")
            for fc in range(4):
                t_.matmul(wps[:], WR1[:, fc, O1 * h: O1 * h + O1], XM[:, fc, :],
                          start=(fc == 0), stop=(fc == 3))
            wsb = SP.tile([O1, R], F32, tag="whmT_sb")
            v_.tensor_copy(wsb[:], wps[:])
            sps = PSB.tile([128, R], F32, tag="ps")
            t_.matmul(sps[:], asrc_rep[(1, h)][:], wsb[:], start=True, stop=True)
            sr1 = P.tile([128, R], F32, tag=f"s_rep_{h}")
            v_.tensor_copy(sr1[:], sps[:])
            s_rep1[h] = sr1

        WhD1 = P.tile([128, NT, 4 * BLK1 + NHEADS], F32, tag="WhD1")
        WhD1v = WhD1[:, :, 0:4 * BLK1].rearrange("p t (h c) -> p t h c", c=BLK1)
        v_.memset(WhD1[:, :, O1: 4 * BLK1: BLK1], 1.0)
        ntph = HN // 128
        for half in range(N // HN):
            for q in range(4):
                w = HN // 4
                nc.sync.dma_start(
                    XT[:, :, w * q: w * (q + 1)],
                    xT[:, HN * half + w * q: HN * half + w * (q + 1)].rearrange(
                        "(c p) n -> p c n", p=128))
            for nt in range(ntph * half, ntph * half + ntph):
                ntl = nt - ntph * half
                fps = PSB.tile([128, 4 * O1 + NHEADS], F32, tag="ps")
                for fc in range(4):
                    t_.matmul(fps[:], XT[:, fc, 128 * ntl: 128 * ntl + 128],
                              WR1[:, fc, :], start=(fc == 0), stop=(fc == 3))
                v_.tensor_copy(
                    WhD1v[:, nt, :, 0:O1],
                    fps[:, 0:4 * O1].rearrange("p (h c) -> p h c", c=O1),
                )
                v_.tensor_copy(WhD1[:, nt, 4 * BLK1: 4 * BLK1 + NHEADS],
                               fps[:, 4 * O1: 4 * O1 + NHEADS])

        if dbg is not None:
            nc.sync.dma_start(dbg["d_srep0"][:, :], s_rep1[0][:])
            nc.sync.dma_start(dbg["d_WhD1"][:, :, :], WhD1[:])
        # ---- layer 1 attention ---------------------------------------
        H1T = P.tile([128, 2, R], F32, tag="H1T")
        _attention(tc, nc, WhD1, s_rep1, adjT, H1T, AP_, VP, SP, PSA, PSB,
                   ones_row, layer=1, Fo=O1, blk=BLK1, dcol=4 * BLK1, dbg=dbg)

        XP_ctx.__exit__(None, None, None)
        LP_ctx = tc.tile_pool(name="late", bufs=1)
        LP = LP_ctx.__enter__()
        if dbg is not None:
            nc.sync.dma_start(dbg["d_H1T"][:, :, :], H1T[:])
        if _on("skip_l2"):
            nc.sync.dma_start(out[:, :], H1T[:, 0, :].rearrange("p i -> p i"))
            LP_ctx.__exit__(None, None, None)
            VP_ctx.__exit__(None, None, None)
            AP_ctx.__exit__(None, None, None)
            return
        # ---- gather phase: Wh2_mine + d2_mine -> AllGather -----------
        W2sb = LP.tile([128, 2, 4 * O2], F32, tag="W2sb")
        for h in range(NHEADS):
            nc.sync.dma_start(
                W2sb[:, :, O2 * h: O2 * h + O2],
                W2[h, :, :].rearrange("(c p) o -> p c o", p=128),
            )
        Gsb = LP.tile([128, 4, NEMBED], F32, tag="Gsb")
        for h in range(NHEADS):
            wh2m = PSB.tile([128, 4, O2], F32, tag="ps", name=f"wh2m_{h}")
            for it in range(4):
                for fc in range(2):
                    t_.matmul(wh2m[:, it, :],
                              H1T[:, fc, 128 * it: 128 * it + 128],
                              W2sb[:, fc, O2 * h: O2 * h + O2],
                              start=(fc == 0), stop=(fc == 1))
            v_.tensor_copy(Gsb[:, :, O2 * h: O2 * h + O2], wh2m[:])
        d2sb = LP.tile([128, 4, NHEADS], F32, tag="d2sb")
        for it in range(4):
            for h in range(NHEADS):
                tmp = SP.tile([128, O2], F32, tag="d2_tmp")
                v_.tensor_tensor(tmp[:], Gsb[:, it, O2 * h: O2 * h + O2],
                                 adst_rep[(2, h)][:], op=OP.mult)
                v_.reduce_sum(d2sb[:, it, h: h + 1], tmp[:],
                              axis=mybir.AxisListType.X)

        # ---- s2_rep ---------------------------------------------------
        s_rep2 = {}
        for h in range(NHEADS):
            wps = PSB.tile([O2, R], F32, tag="ps")
            for fc in range(2):
                t_.matmul(wps[:], W2sb[:, fc, O2 * h: O2 * h + O2],
                          H1T[:, fc, :], start=(fc == 0), stop=(fc == 1))
            wsb = SP.tile([O2, R], F32, tag="whmT_sb")
            v_.tensor_copy(wsb[:], wps[:])
            sps = PSB.tile([128, R], F32, tag="ps")
            t_.matmul(sps[:], asrc_rep[(2, h)][:], wsb[:], start=True, stop=True)
            srt = P.tile([128, R], F32, tag=f"s_rep_{h}")
            v_.tensor_copy(srt[:], sps[:])
            s_rep2[h] = srt

        Gmine = DP.tile([R, GCOLS], F32, tag="Gmine")
        Gall = DP.tile([N, GCOLS], F32, tag="Gall", addr_space="Shared")
        nc.sync.dma_start(
            Gmine[:, 0:NEMBED].rearrange("(t p) o -> p t o", p=128), Gsb[:])
        nc.sync.dma_start(
            Gmine[:, NEMBED:GCOLS].rearrange("(t p) o -> p t o", p=128), d2sb[:])
        if _on("skip_gather"):
            nc.sync.dma_start(Gall[0:R, :], Gmine[:, :])
        else:
            nc.gpsimd.collective_compute(
                "AllGather", OP.bypass,
                replica_groups=[list(range(NCORES))],
                ins=[Gmine[:].opt()], outs=[Gall[:].opt()],
            )

        WhD2 = LP.tile([128, NT, 4 * BLK2 + NHEADS], F32, tag="WhD2")
        v_.memset(WhD2[:, :, O2: 4 * BLK2: BLK2], 1.0)
        nwq = 1 if _on("no_whd2_split") else 4
        for h in range(NHEADS):
            for q in range(nwq):
                tw = NT // nwq
                nc.sync.dma_start(
                    WhD2[:, tw * q: tw * (q + 1), BLK2 * h: BLK2 * h + O2],
                    Gall[128 * tw * q: 128 * tw * (q + 1),
                         O2 * h: O2 * h + O2].rearrange("(t p) o -> p t o", p=128),
                )
        for q in range(nwq):
            tw = NT // nwq
            nc.sync.dma_start(
                WhD2[:, tw * q: tw * (q + 1), 4 * BLK2: 4 * BLK2 + NHEADS],
                Gall[128 * tw * q: 128 * tw * (q + 1),
                     NEMBED:GCOLS].rearrange("(t p) o -> p t o", p=128),
            )

        if dbg is not None:
            nc.sync.dma_start(dbg["d_Gall"][:, :], Gall[:])

        if dbg is not None:
            nc.sync.dma_start(dbg["d_srep20"][:, :], s_rep2[0][:])
        # ---- layer 2 attention ---------------------------------------
        H2T = LP.tile([NEMBED, R], F32, tag="H2T")
        _attention(tc, nc, WhD2, s_rep2, adjT, H2T, AP_, VP, SP, PSA, PSB,
                   ones_row, layer=2, Fo=O2, blk=BLK2, dcol=4 * BLK2)

        nc.sync.dma_start(out[:, :], H2T[:])
        LP_ctx.__exit__(None, None, None)
        VP_ctx.__exit__(None, None, None)
        AP_ctx.__exit__(None, None, None)


def _attention(tc, nc, WhD, s_rep, adjT, Hout, AP_, VP, SP, PSA, PSB,
               ones_row, layer, Fo, blk, dcol, dbg=None):
    v_ = nc.vector
    s_ = nc.scalar
    t_ = nc.tensor
    sb_sz = VARIANT.get("sb", SB)
    nsb = NT // sb_sz

    if _on("skip_att"):
        v_.memset(Hout[:], 0.5)
        return
    hT = [PSA.tile([Fo + 1, R], F32, tag=f"hT_{h}", name=f"hT_{layer}_{h}")
          for h in range(NHEADS)]
    with nc.named_scope(f"att_l{layer}"):
        for b in range(nsb):
            j0 = sb_sz * 128 * b
            adj_t = AP_.tile([128, sb_sz, R], F32, tag="adj")
            nq = 1 if (_on("no_split_dma") or sb_sz <= 4) else \
                VARIANT.get("adj_nq", 2)
            for q in range(nq):
                w = sb_sz // nq
                eng = (nc.scalar if (_on("dma_spread") and (b + q) % 2)
                       else nc.sync)
                eng.dma_start(
                    adj_t[:, w * q: w * (q + 1), :],
                    adjT[j0 + 128 * w * q: j0 + 128 * w * (q + 1),
                         :].rearrange("(t p) i -> p t i", p=128),
                )
            for h in range(NHEADS):
                v = VP.tile([128, sb_sz, R], F32, tag="v")
                if _on("skip_ts"):
                    v_.tensor_copy(v[:, 0, :], s_rep[h][:])
                elif _on("ts_bcast"):
                    sb_ap = s_rep[h][:].rearrange(
                        "p (o i) -> p o i", o=1).to_broadcast((128, sb_sz, R))
                    db_ap = WhD[:, sb_sz * b: sb_sz * b + sb_sz,
                                dcol + h: dcol + h + 1].to_broadcast((128, sb_sz, R))
                    v_.tensor_tensor(v[:, :, :], sb_ap, db_ap, op=OP.add)
                else:
                    for t in range(sb_sz):
                        jt = sb_sz * b + t
                        v_.tensor_scalar(v[:, t, :], s_rep[h][:],
                                         WhD[:, jt, dcol + h: dcol + h + 1], None,
                                         op0=OP.add)
                hs = sb_sz if _on("act_whole") else sb_sz // 2
                for ph in range(sb_sz // hs):
                    vv = v[:, hs * ph: hs * (ph + 1), :]
                    aa = adj_t[:, hs * ph: hs * (ph + 1), :]
                    if not _on("skip_prelu"):
                        s_.activation(vv, vv, AF.Prelu,
                                      bias=0.0, scale=1.0, alpha=ALPHA)
                    if not _on("skip_exp"):
                        s_.activation(vv, vv, AF.Exp)
                    if not _on("skip_mask"):
                        eng = (nc.gpsimd if h < VARIANT.get("gp_mask_heads", 0)
                               else v_)
                        eng.tensor_tensor(vv, vv, aa, op=OP.mult)
                if dbg is not None and layer == 1 and b == 0 and h == 0 \
                        and sb_sz == SB:
                    nc.sync.dma_start(dbg["d_att0"][:, :, :], v[:, :, :])
                if not _on("skip_mm"):
                    for t in range(sb_sz):
                        jt = sb_sz * b + t
                        t_.matmul(hT[h][:], WhD[:, jt, blk * h: blk * h + blk],
                                  v[:, t, :], start=(jt == 0), stop=(jt == NT - 1))
                    if _on("tails_inline") and b == nsb - 1 \
                            and not _on("skip_tails"):
                        _emit_tail(nc, SP, PSB, ones_row, hT, Hout, layer, Fo, h)
                elif b == nsb - 1:
                    t_.matmul(hT[h][:], WhD[:, 0, blk * h: blk * h + blk],
                              v[:, 0, :], start=True, stop=True)

    if _on("skip_tails"):
        for h in range(NHEADS):
            if layer == 1:
                dst = Hout[64 * (h % 2): 64 * (h % 2) + 64, h // 2, :]
            else:
                dst = Hout[O2 * h: O2 * h + O2, :]
            v_.tensor_copy(dst, hT[h][0:Fo, :])
        return
    if not _on("tails_inline"):
        with nc.named_scope(f"tail_l{layer}"):
            for h in range(NHEADS):
                _emit_tail(nc, SP, PSB, ones_row, hT, Hout, layer, Fo, h)


VARIANT = {}


def _on(flag):
    return VARIANT.get(flag, False)


def _emit_tail(nc, SP, PSB, ones_row, hT, Hout, layer, Fo, h):
    v_ = nc.vector
    s_ = nc.scalar
    t_ = nc.tensor
    r1 = SP.tile([1, R], F32, tag="recip", name=f"r1_{layer}_{h}")
    v_.reciprocal(r1[:], hT[h][Fo: Fo + 1, :])
    rps = PSB.tile([128, R], F32, tag="ps", name=f"rps_{layer}_{h}")
    t_.matmul(rps[:], ones_row[:], r1[:], start=True, stop=True)
    rrep = SP.tile([128, R], F32, tag="rrep", name=f"rrep_{layer}_{h}")
    v_.tensor_copy(rrep[:], rps[:])
    hn = SP.tile([Fo, R], F32, tag="hn", name=f"hn_{layer}_{h}")
    v_.tensor_tensor(hn[:], hT[h][0:Fo, :], rrep[0:Fo, :], op=OP.mult)
    # ELU(x) = max(x,0) - 1 + exp(min(x,0))
    m = SP.tile([Fo, R], F32, tag="elu_m", name=f"m_{layer}_{h}")
    v_.tensor_scalar(m[:], hn[:], 0.0, None, op0=OP.min)
    s_.activation(m[:], m[:], AF.Exp)
    rl = SP.tile([Fo, R], F32, tag="elu_rl", name=f"rl_{layer}_{h}")
    v_.tensor_scalar(rl[:], hn[:], 0.0, -1.0, op0=OP.max, op1=OP.add)
    if layer == 1:
        dst = Hout[64 * (h % 2): 64 * (h % 2) + 64, h // 2, :]
    else:
        dst = Hout[O2 * h: O2 * h + O2, :]
    v_.tensor_tensor(dst, m[:], rl[:], op=OP.add)


_NC_CACHE = {}


def _get_nc():
    if "nc" not in _NC_CACHE:
        _NC_CACHE["nc"] = _build()
    return _NC_CACHE["nc"]


def _in_maps_for(inputs):
    x = np.ascontiguousarray(np.asarray(inputs["x"], dtype=np.float32))
    adj = np.asarray(inputs["adj"], dtype=np.float32)
    xT = np.ascontiguousarray(x.T)
    a1f = np.ascontiguousarray(np.asarray(inputs["a1"], np.float32).reshape(1, -1))
    a2f = np.ascontiguousarray(np.asarray(inputs["a2"], np.float32).reshape(1, -1))
    W1c = np.ascontiguousarray(np.asarray(inputs["W1"], np.float32))
    W2c = np.ascontiguousarray(np.asarray(inputs["W2"], np.float32))

    in_maps = []
    for c in range(NCORES):
        rows = slice(R * c, R * (c + 1))
        in_maps.append({
            "xT": xT,
            "xmT": np.ascontiguousarray(x[rows, :].T),
            "adjT": np.ascontiguousarray(adj[rows, :].T),
            "W1": W1c, "a1f": a1f, "W2": W2c, "a2f": a2f,
        })
    return in_maps


def kernel(x, adj, W1, a1, W2, a2):
    nc = _get_nc()
    in_maps = _in_maps_for(dict(x=x, adj=adj, W1=W1, a1=a1, W2=W2, a2=a2))
    res = run_bass_kernel_spmd(nc, in_maps, core_ids=list(range(NCORES)))
    return np.concatenate(
        [np.asarray(res.results[c]["h2T"]).T for c in range(NCORES)], axis=0)

